# revision 15
# baseline (speedup 1.0000x reference)
"""Causal depthwise conv (B=8, L=4096, D=1024, K=15) on 8 TRN2 NeuronCores.

Sharding: channels split across the 8 cores (128 channels each); every core
processes all 8 batch sequences for its channel slice. Host re-lays-out x to
[channels, batch, time] fp16 so on-chip tiles have channels on SBUF
partitions and time on the free dimension; tap shifts are free-dim offsets.

Engine split of the 15 taps (fp16 compute, fp32 PSUM accumulation). All
three compute engines end up ~85-95% busy; measured per-op costs on HW:
PE matmul ~230-240ns per FD-512 (incl. shared-SBUF contention), DVE
tensor_scalar 4x-mode mul ~1.3us / tensor_tensor 2x-mode add ~2.2us per
4096-wide op, ScalarE activation-mul ~3.8us, PSUM bridge copy ~1.9us:
  - TensorE (10 taps {0,1,3,5,7,9,11,12,13,14}): diagonal-weight matmuls
    accumulating into 2048-wide PSUM halves; ScalarE bridges PSUM->SBUF
    fp16 (keeps DVE decoupled from the PE tail).
  - ScalarE (2 taps {8,10}): activation-mul products (per-partition
    scale), plus the two bridge copies per batch.
  - DVE (taps {2,4,6} + all accumulation): tensor_scalar_mul products (4x
    packed mode - offsets must stay even and 4B-aligned), then a fold
    chain of five 4096-wide tensor_tensor adds (2x mode) ending in the
    merge with the bridged PE partial.
Batch 0 staggers its x DMA into pieces (and runs the first PSUM half
q-outer) so the PE starts ~1us in; the last batch runs a 1024-chunked
bridge/merge/store epilogue to shorten the serial tail. Output is written
fp16; the host upcasts to fp32 (rel err ~4.6e-4 total vs fp32 reference).
"""

from contextlib import ExitStack

import numpy as np

import concourse.bacc as bacc
import concourse.tile as tile
from concourse import mybir
from concourse.bass_utils import run_bass_kernel_spmd

F32 = mybir.dt.float32
F16 = mybir.dt.float16
F16NP = np.float16

B = 8
L = 4096
D = 1024
K = 15
NCORES = 8
CPC = D // NCORES  # channels per core = 128
LP = L + K - 1  # 4110

DVE_MUL_TAPS = [2, 4, 6]  # even offsets -> DVE 4x packed mode stays legal
SC_MUL_TAPS = [8, 10]
PE_TAPS = [0, 1, 3, 5, 7, 9, 11, 12, 13, 14]

_compiled_nc = None
_last_in_maps = None


def _build_nc():
    nc = bacc.Bacc(
        "TRN2",
        target_bir_lowering=False,
        debug=False,
        enable_asserts=True,
        num_devices=NCORES,
    )
    x = nc.dram_tensor("x", [CPC, B, LP], F16, kind="ExternalInput").ap()
    diag = nc.dram_tensor("diag", [len(PE_TAPS), CPC, CPC], F16, kind="ExternalInput").ap()
    w = nc.dram_tensor("w", [CPC, 16], F32, kind="ExternalInput").ap()
    out = nc.dram_tensor("out", [CPC, B, L], F16, kind="ExternalOutput").ap()

    add = mybir.AluOpType.add

    with tile.TileContext(nc) as tc, ExitStack() as ctx:
        const_pool = ctx.enter_context(tc.tile_pool(name="const", bufs=1))
        xp = ctx.enter_context(tc.tile_pool(name="xp", bufs=4))
        prodp = ctx.enter_context(tc.tile_pool(name="prodp", bufs=5))
        sump = ctx.enter_context(tc.tile_pool(name="sump", bufs=7))
        accp = ctx.enter_context(tc.tile_pool(name="accp", bufs=2))
        op = ctx.enter_context(tc.tile_pool(name="op", bufs=2))
        pp = ctx.enter_context(tc.tile_pool(name="pp", bufs=2, space="PSUM"))

        # Startup order: first x(b0) piece, then the PE diag weights, then
        # the rest of x(b0) - the sync ring front-loads what the first
        # matmuls need; the scalar ring takes x(b0)'s tail in parallel.
        xt0 = xp.tile([CPC, LP], F16, tag="x", name="x_0")
        nc.sync.dma_start(xt0[:, 0:600], x[:, 0, 0:600])
        dg = const_pool.tile([CPC, len(PE_TAPS) * CPC], F16, tag="diag")
        for j in range(len(PE_TAPS)):
            nc.sync.dma_start(dg[:, j * CPC : (j + 1) * CPC], diag[j])
        nc.scalar.dma_start(xt0[:, 2900:LP], x[:, 0, 2900:LP])
        for s0, s1 in [(600, 1300), (1300, 2100), (2100, 2900)]:
            nc.sync.dma_start(xt0[:, s0:s1], x[:, 0, s0:s1])
        wt = const_pool.tile([CPC, 16], F32, tag="w")
        nc.sync.dma_start(wt[:], w[:])

        for b in range(B):
            if b == 0:
                xt = xt0
            else:
                xt = xp.tile([CPC, LP], F16, tag="x", name=f"x_{b}")
                for s0, s1 in [(0, LP // 2), (LP // 2, LP)]:
                    nc.sync.dma_start(xt[:, s0:s1], x[:, b, s0:s1])

            # ScalarE products for taps {8,10}
            prods = {}
            for k in SC_MUL_TAPS:
                pt = prodp.tile([CPC, L], F16, tag="prod", name=f"sp_{b}_{k}")
                nc.scalar.mul(pt[:], xt[:, k : k + L], wt[:, k : k + 1])
                prods[k] = pt

            # TensorE: 10 taps into PSUM, two 2048-wide halves + ScalarE bridge
            last = b == B - 1
            acc = accp.tile([CPC, L], F16, tag="acc", name=f"acc_{b}")
            for h in range(2):
                t0 = h * 2048
                ps = pp.tile([CPC, 2048], F32, tag="ps", name=f"ps_{b}_{h}")
                if b == 0 and h == 0:
                    # q-outer: match the PE start to the x DMA arrival pace
                    loop = [(ji, k, q) for q in range(4) for ji, k in enumerate(PE_TAPS)]
                else:
                    loop = [(ji, k, q) for ji, k in enumerate(PE_TAPS) for q in range(4)]
                for ji, k, q in loop:
                    nc.tensor.matmul(
                        ps[:, q * 512 : (q + 1) * 512],
                        dg[:, ji * CPC : (ji + 1) * CPC],
                        xt[:, t0 + k + q * 512 : t0 + k + (q + 1) * 512],
                        start=(ji == 0),
                        stop=(ji == len(PE_TAPS) - 1),
                    )
                if last:
                    for q in range(2):
                        nc.scalar.copy(
                            acc[:, t0 + q * 1024 : t0 + (q + 1) * 1024],
                            ps[:, q * 1024 : (q + 1) * 1024],
                        )
                else:
                    nc.scalar.copy(acc[:, t0 : t0 + 2048], ps[:])

            # DVE: products for taps {2,4} (4x mode), fold chain, merge
            for k in DVE_MUL_TAPS:
                pt = prodp.tile([CPC, L], F16, tag="prod", name=f"dp_{b}_{k}")
                nc.vector.tensor_scalar_mul(pt[:], xt[:, k : k + L], wt[:, k : k + 1])
                prods[k] = pt
            s = prods[2]
            for i, k in enumerate([4, 6, 8, 10]):
                dst = sump.tile([CPC, L], F16, tag="sum", name=f"s_{b}_{i}")
                nc.vector.tensor_tensor(dst[:], prods[k][:], s[:], add)
                s = dst
            ot = op.tile([CPC, L], F16, tag="osb", name=f"o_{b}")
            if last:
                for c in range(4):
                    sl = slice(c * 1024, (c + 1) * 1024)
                    nc.vector.tensor_tensor(ot[:, sl], s[:, sl], acc[:, sl], add)
                    nc.scalar.dma_start(out[:, b, sl], ot[:, sl])
            else:
                nc.vector.tensor_tensor(ot[:], s[:], acc[:], add)
                nc.scalar.dma_start(out[:, b, :], ot[:])

    nc.compile()
    return nc


def kernel(x: np.ndarray, weight: np.ndarray) -> np.ndarray:
    """x: [8, 4096, 1024] fp32, weight: [15, 1, 1024] fp32 ->
    [8, 4096, 1024] fp32 causal depthwise conv."""
    global _compiled_nc
    if _compiled_nc is None:
        _compiled_nc = _build_nc()
    nc = _compiled_nc

    x = np.ascontiguousarray(x, dtype=np.float32)
    wk = np.ascontiguousarray(weight, dtype=np.float32).reshape(K, D)
    x16 = x.astype(F16NP)
    wk16 = wk.astype(F16NP)

    in_maps = []
    for c in range(NCORES):
        sl = slice(c * CPC, (c + 1) * CPC)
        xpad = np.zeros((CPC, B, LP), dtype=F16NP)
        xpad[:, :, K - 1 :] = x16[:, :, sl].transpose(2, 0, 1)
        dgc = np.zeros((len(PE_TAPS), CPC, CPC), dtype=F16NP)
        didx = np.arange(CPC)
        for j, k in enumerate(PE_TAPS):
            dgc[j, didx, didx] = wk16[k, sl]
        wt = np.zeros((CPC, 16), dtype=np.float32)
        wt[:, :K] = wk[:, sl].T
        in_maps.append({"x": xpad, "diag": dgc, "w": wt})

    global _last_in_maps
    _last_in_maps = in_maps
    res = run_bass_kernel_spmd(nc, in_maps, list(range(NCORES)))

    out = np.empty((B, L, D), dtype=np.float32)
    for c in range(NCORES):
        sl = slice(c * CPC, (c + 1) * CPC)
        out[:, :, sl] = res.results[c]["out"].transpose(1, 2, 0).astype(np.float32)
    return out


# revision 17
# speedup vs baseline: 1.0461x; 1.0461x over previous
"""Causal depthwise conv (B=8, L=4096, D=1024, K=15) on 8 TRN2 NeuronCores.

Sharding: channels split across the 8 cores (128 channels each); every core
processes all 8 batch sequences for its channel slice. Host re-lays-out x to
[channels, batch, time] fp16 so on-chip tiles have channels on SBUF
partitions and time on the free dimension; tap shifts are free-dim offsets.

Engine split of the 15 taps (fp16 compute, fp32 PSUM accumulation). All
three compute engines end up ~85-95% busy; measured per-op costs on HW:
PE matmul ~230-240ns per FD-512 (incl. shared-SBUF contention), DVE
tensor_scalar 4x-mode mul ~1.3us / tensor_tensor 2x-mode add ~2.2us per
4096-wide op, ScalarE activation-mul ~3.8us, PSUM bridge copy ~1.9us:
  - TensorE (10 taps {0,1,3,5,7,9,11,12,13,14}): diagonal-weight matmuls
    accumulating into 2048-wide PSUM halves; ScalarE bridges PSUM->SBUF
    fp16 (keeps DVE decoupled from the PE tail).
  - ScalarE (2 taps {8,10}): activation-mul products (per-partition
    scale), plus the two bridge copies per batch.
  - DVE (taps {2,4,6} + all accumulation): tensor_scalar_mul products (4x
    packed mode - offsets must stay even and 4B-aligned), then a fold
    chain of five 4096-wide tensor_tensor adds (2x mode) ending in the
    merge with the bridged PE partial.
Batch 0 staggers its x DMA into pieces (and runs the first PSUM half
q-outer) so the PE starts ~1us in; the last batch runs a 1024-chunked
bridge/merge/store epilogue to shorten the serial tail. Output is written
fp16; the host upcasts to fp32 (rel err ~4.6e-4 total vs fp32 reference).
"""

from contextlib import ExitStack

import numpy as np

import concourse.bacc as bacc
import concourse.tile as tile
from concourse import mybir
from concourse.bass_utils import run_bass_kernel_spmd

F32 = mybir.dt.float32
F16 = mybir.dt.float16
F16NP = np.float16

B = 8
L = 4096
D = 1024
K = 15
NCORES = 8
CPC = D // NCORES  # channels per core = 128
LP = L + K - 1  # 4110

DVE_MUL_TAPS = [2, 4, 6]  # even offsets -> DVE 4x packed mode stays legal
SC_MUL_TAPS = [8, 10]
PE_TAPS = [0, 1, 3, 5, 7, 9, 11, 12, 13, 14]

_compiled_nc = None
_last_in_maps = None


def _build_nc():
    nc = bacc.Bacc(
        "TRN2",
        target_bir_lowering=False,
        debug=False,
        enable_asserts=True,
        num_devices=NCORES,
    )
    x = nc.dram_tensor("x", [CPC, B, LP], F16, kind="ExternalInput").ap()
    diag = nc.dram_tensor("diag", [len(PE_TAPS), CPC, CPC], F16, kind="ExternalInput").ap()
    w = nc.dram_tensor("w", [CPC, 16], F32, kind="ExternalInput").ap()
    out = nc.dram_tensor("out", [CPC, B, L], F16, kind="ExternalOutput").ap()

    add = mybir.AluOpType.add

    with tile.TileContext(nc) as tc, ExitStack() as ctx:
        const_pool = ctx.enter_context(tc.tile_pool(name="const", bufs=1))
        xp = ctx.enter_context(tc.tile_pool(name="xp", bufs=4))
        prodp = ctx.enter_context(tc.tile_pool(name="prodp", bufs=5))
        sump = ctx.enter_context(tc.tile_pool(name="sump", bufs=7))
        accp = ctx.enter_context(tc.tile_pool(name="accp", bufs=2))
        op = ctx.enter_context(tc.tile_pool(name="op", bufs=2))
        php = ctx.enter_context(tc.tile_pool(name="php", bufs=3))
        pp = ctx.enter_context(tc.tile_pool(name="pp", bufs=2, space="PSUM"))

        # Startup order: first x(b0) piece, then the PE diag weights, then
        # the rest of x(b0) - the sync ring front-loads what the first
        # matmuls need; the scalar ring takes x(b0)'s tail in parallel.
        xt0 = xp.tile([CPC, LP], F16, tag="x", name="x_0")
        nc.sync.dma_start(xt0[:, 0:600], x[:, 0, 0:600])
        dg = const_pool.tile([CPC, len(PE_TAPS) * CPC], F16, tag="diag")
        for j in range(len(PE_TAPS)):
            nc.sync.dma_start(dg[:, j * CPC : (j + 1) * CPC], diag[j])
        nc.scalar.dma_start(xt0[:, 2900:LP], x[:, 0, 2900:LP])
        for s0, s1 in [(600, 1300), (1300, 2100), (2100, 2900)]:
            nc.sync.dma_start(xt0[:, s0:s1], x[:, 0, s0:s1])
        wt = const_pool.tile([CPC, 16], F32, tag="w")
        nc.sync.dma_start(wt[:], w[:])

        for b in range(B):
            if b == 0:
                xt = xt0
            else:
                xt = xp.tile([CPC, LP], F16, tag="x", name=f"x_{b}")
                for s0, s1 in [(0, LP // 2), (LP // 2, LP)]:
                    nc.sync.dma_start(xt[:, s0:s1], x[:, b, s0:s1])

            # ScalarE products: taps {8,10} full + tap 12 second half
            # (tap 12's first half stays on the PE)
            prods = {}
            for k in SC_MUL_TAPS:
                pt = prodp.tile([CPC, L], F16, tag="prod", name=f"sp_{b}_{k}")
                nc.scalar.mul(pt[:], xt[:, k : k + L], wt[:, k : k + 1])
                prods[k] = pt
            p12 = php.tile([CPC, 2048], F16, tag="prodh", name=f"sp12_{b}")
            nc.scalar.mul(p12[:], xt[:, 2048 + 12 : 2048 + 12 + 2048], wt[:, 12:13])

            # TensorE: 10 taps into PSUM, two 2048-wide halves + ScalarE bridge
            last = b == B - 1
            acc = accp.tile([CPC, L], F16, tag="acc", name=f"acc_{b}")
            for h in range(2):
                t0 = h * 2048
                ps = pp.tile([CPC, 2048], F32, tag="ps", name=f"ps_{b}_{h}")
                taps_h = PE_TAPS if h == 0 else [k for k in PE_TAPS if k != 12]
                if b == 0 and h == 0:
                    # q-outer: match the PE start to the x DMA arrival pace
                    loop = [(ji, k, q) for q in range(4) for ji, k in enumerate(taps_h)]
                else:
                    loop = [(ji, k, q) for ji, k in enumerate(taps_h) for q in range(4)]
                for ji, k, q in loop:
                    nc.tensor.matmul(
                        ps[:, q * 512 : (q + 1) * 512],
                        dg[:, PE_TAPS.index(k) * CPC : (PE_TAPS.index(k) + 1) * CPC],
                        xt[:, t0 + k + q * 512 : t0 + k + (q + 1) * 512],
                        start=(ji == 0),
                        stop=(ji == len(taps_h) - 1),
                    )
                if last:
                    for q in range(2):
                        nc.scalar.copy(
                            acc[:, t0 + q * 1024 : t0 + (q + 1) * 1024],
                            ps[:, q * 1024 : (q + 1) * 1024],
                        )
                else:
                    nc.scalar.copy(acc[:, t0 : t0 + 2048], ps[:])

            # DVE: products for taps {2,4} (4x mode), fold chain, merge
            for k in DVE_MUL_TAPS:
                pt = prodp.tile([CPC, L], F16, tag="prod", name=f"dp_{b}_{k}")
                nc.vector.tensor_scalar_mul(pt[:], xt[:, k : k + L], wt[:, k : k + 1])
                prods[k] = pt
            s = prods[2]
            for i, k in enumerate([4, 6, 8, 10]):
                dst = sump.tile([CPC, L], F16, tag="sum", name=f"s_{b}_{i}")
                nc.vector.tensor_tensor(dst[:], prods[k][:], s[:], add)
                s = dst
            s5 = php.tile([CPC, 2048], F16, tag="sumh", name=f"s5_{b}")
            nc.vector.tensor_tensor(s5[:], s[:, 2048:4096], p12[:], add)
            ot = op.tile([CPC, L], F16, tag="osb", name=f"o_{b}")
            if last:
                for c in range(2):
                    sl = slice(c * 1024, (c + 1) * 1024)
                    nc.vector.tensor_tensor(ot[:, sl], s[:, sl], acc[:, sl], add)
                    nc.scalar.dma_start(out[:, b, sl], ot[:, sl])
                for c in range(2):
                    sl = slice(2048 + c * 1024, 2048 + (c + 1) * 1024)
                    sh = slice(c * 1024, (c + 1) * 1024)
                    nc.vector.tensor_tensor(ot[:, sl], s5[:, sh], acc[:, sl], add)
                    nc.scalar.dma_start(out[:, b, sl], ot[:, sl])
            else:
                nc.vector.tensor_tensor(ot[:, 0:2048], s[:, 0:2048], acc[:, 0:2048], add)
                nc.vector.tensor_tensor(ot[:, 2048:4096], s5[:], acc[:, 2048:4096], add)
                nc.scalar.dma_start(out[:, b, :], ot[:])

    nc.compile()
    return nc


def kernel(x: np.ndarray, weight: np.ndarray) -> np.ndarray:
    """x: [8, 4096, 1024] fp32, weight: [15, 1, 1024] fp32 ->
    [8, 4096, 1024] fp32 causal depthwise conv."""
    global _compiled_nc
    if _compiled_nc is None:
        _compiled_nc = _build_nc()
    nc = _compiled_nc

    x = np.ascontiguousarray(x, dtype=np.float32)
    wk = np.ascontiguousarray(weight, dtype=np.float32).reshape(K, D)
    x16 = x.astype(F16NP)
    wk16 = wk.astype(F16NP)

    in_maps = []
    for c in range(NCORES):
        sl = slice(c * CPC, (c + 1) * CPC)
        xpad = np.zeros((CPC, B, LP), dtype=F16NP)
        xpad[:, :, K - 1 :] = x16[:, :, sl].transpose(2, 0, 1)
        dgc = np.zeros((len(PE_TAPS), CPC, CPC), dtype=F16NP)
        didx = np.arange(CPC)
        for j, k in enumerate(PE_TAPS):
            dgc[j, didx, didx] = wk16[k, sl]
        wt = np.zeros((CPC, 16), dtype=np.float32)
        wt[:, :K] = wk[:, sl].T
        in_maps.append({"x": xpad, "diag": dgc, "w": wt})

    global _last_in_maps
    _last_in_maps = in_maps
    res = run_bass_kernel_spmd(nc, in_maps, list(range(NCORES)))

    out = np.empty((B, L, D), dtype=np.float32)
    for c in range(NCORES):
        sl = slice(c * CPC, (c + 1) * CPC)
        out[:, :, sl] = res.results[c]["out"].transpose(1, 2, 0).astype(np.float32)
    return out
